# revision 1
# baseline (speedup 1.0000x reference)
"""BinaryConnect 3x3 SAME conv (NHWC, 32x112x112x128 -> 32x112x112x256) on 8 trn2 cores.

Strategy (data-parallel, 4 images per core):
  - Host: binarize kernel to +/-1 fp16 (exact), cast x to fp16, transpose to
    channel-major [cin, n, hp, wp] with a 1-px zero halo (115x114 rows incl.
    one zero tail row).
  - Device: for each output tile of 4 rows x 112 cols (one cout half), the
    conv is 9 accumulating matmuls (one per 3x3 tap):
      lhsT = wb[tap] [cin=128, cout_half=128]   (stationary),
      rhs  = x[cin=128, rows r0+dh : r0+dh+4, cols dw : dw+112] (2D-AP moving,
             N = 448), fp16 in, fp32 PSUM accumulate.
    Output is written channel-major [cout, n, h*112+w] and un-transposed on
    the host. PE warmup matmuls on a memset tile un-throttle the HAM clock
    gate while the first input band DMA is still in flight.
"""

import os

import numpy as np

import concourse.bass as bass
import concourse.mybir as mybir
import concourse.tile as tile
from concourse import bacc
from concourse.bass_utils import run_bass_kernel_spmd

N_CORES = 8
NPC = 4            # images per core
H = 112
WP = 114           # padded row width
HP = 115           # 1 top pad + 112 rows + 1 bottom pad + 1 zero tail row
CI = 128
CO = 256
TROWS = 4          # output rows per matmul tile
S = TROWS * H      # 448 matmul free dim (<=512 fp32 PSUM bank)
BROWS = 28         # output rows per input band
NB = H // BROWS    # 4 bands per image
BIN = BROWS + 3    # input rows per band incl. halo
TSB = BROWS // TROWS  # 7 tiles per band

_nc_cache = None
LAST_RESULT = None


def _build():
    nc = bacc.Bacc(
        "TRN2",
        target_bir_lowering=False,
        debug=False,
        num_devices=N_CORES,
    )
    x_d = nc.dram_tensor(
        "xp", [CI, NPC, HP, WP], mybir.dt.float16, kind="ExternalInput"
    )
    w_d = nc.dram_tensor(
        "wt", [CI, 2, 9 * 128], mybir.dt.float16, kind="ExternalInput"
    )
    o_d = nc.dram_tensor(
        "out_cm", [CO, NPC, H * H], mybir.dt.float32, kind="ExternalOutput"
    )
    with tile.TileContext(nc) as tc:
        with (
            tc.tile_pool(name="xpool", bufs=1) as xpool,
            tc.tile_pool(name="wpool", bufs=1) as wpool,
            tc.tile_pool(name="psum", bufs=8, space=bass.MemorySpace.PSUM) as psum,
            tc.tile_pool(name="opool", bufs=12) as opool,
        ):
            # Warmup operand with no DMA dependency: memset, so the PE warmup
            # (HAM un-throttle) can start right after the framework preamble,
            # overlapping the input DMA latency.
            wta = wpool.tile([CI, S], mybir.dt.float16, tag="wta", name="wta")
            nc.gpsimd.memset(wta[:], 0.0)
            # Weights split by cout half (separate tiles, so the first matmul
            # group gates on only a 295KB DMA); first-chunk input in between.
            wt0 = wpool.tile([CI, 9 * 128], mybir.dt.float16, tag="wt0", name="wt0")
            nc.sync.dma_start(wt0[:], w_d[:, 0, :])
            # Small first chunk of image 0 (rows 0-7) so the first real
            # matmul group (st=0) gates on ~230KB instead of a full band.
            # On the ACT ring: completion receipts serialize per HWDGE ring,
            # so keeping xa off the sync ring lets its sem fire independently
            # of the weight DMAs'.
            xa = xpool.tile([CI, 12, WP], mybir.dt.float16, tag="xa", name="xa")
            nc.scalar.dma_start(xa[:], x_d[:, 0, 0:12, :])
            wt1 = wpool.tile([CI, 9 * 128], mybir.dt.float16, tag="wt1", name="wt1")
            nc.sync.dma_start(wt1[:], w_d[:, 1, :])
            wt_h = [wt0, wt1]
            # PE warmup: 9 throwaway matmuls to push the HAM activity window
            # to K=8/8 before the real stream begins.
            wu = psum.tile([128, S], mybir.dt.float32, name="ps")
            for _ in range(9):
                nc.tensor.matmul(
                    wu[:], wta[:, 0:128], wta[:, 0:S], start=True, stop=True
                )
            # Image 0 is band-split (4 bands of 28 output rows, 31 input rows
            # each incl. halo) so early compute gates on ~900KB chunks.
            # Images 1-3 arrive long before they're needed, so they load as
            # one DMA each — fewer semaphores shortens the kernel-tail
            # drain/reset cascade and the per-ring receipt chain.
            xs = {}
            for b in range(NB):
                xt = xpool.tile(
                    [CI, BIN, WP], mybir.dt.float16, tag=f"x0_{b}", name=f"x0_{b}"
                )
                nc.sync.dma_start(xt[:], x_d[:, 0, b * BROWS : b * BROWS + BIN, :])
                xs[0, b] = xt
            for n in range(1, NPC):
                xt = xpool.tile(
                    [CI, HP, WP], mybir.dt.float16, tag=f"xi{n}", name=f"xi{n}"
                )
                nc.sync.dma_start(xt[:], x_d[:, n, :, :])
                for b in range(NB):
                    xs[n, b] = xt
            # Spatial tiles are processed in pairs per output DMA: one DMA
            # covering 2 tiles doubles the per-partition contiguous run
            # (1.8KB -> 3.6KB packets), halving the SDMA packet count the
            # output queue must drain (it otherwise backlogs ~8us at the end).
            pairs = [(0, 1), (2, 3), (4, 5), (6,)]

            def emit_group(n, b, st, half, ot, j):
                if n == 0:
                    r0 = st * TROWS  # band-relative top output row
                    xsrc = xa if b == 0 and st <= 1 else xs[n, b]
                else:
                    r0 = b * BROWS + st * TROWS  # image-relative row
                    xsrc = xs[n, b]
                ps = psum.tile([128, S], mybir.dt.float32, name="ps")
                t = 0
                for dh in range(3):
                    for dw in range(3):
                        nc.tensor.matmul(
                            ps[:],
                            wt_h[half][:, t * 128 : t * 128 + 128],
                            xsrc[:, r0 + dh : r0 + dh + TROWS, dw : dw + H],
                            start=(t == 0),
                            stop=(t == 8),
                        )
                        t += 1
                nc.vector.tensor_copy(ot[:, j * S : (j + 1) * S], ps[:])

            def emit_dma(n, b, half, sts, ot):
                width = len(sts) * S
                o0 = (b * BROWS + sts[0] * TROWS) * H
                # ACT's HWDGE ring — keeps output DMAs off the sync ring so
                # they don't queue behind input DMAs.
                nc.scalar.dma_start(
                    o_d[half * 128 : half * 128 + 128, n, o0 : o0 + width],
                    ot[:, 0:width],
                )

            for n in range(NPC):
                for b in range(NB):
                    for sts in pairs:
                        if (n, b, sts) == (0, 0, (0, 1)):
                            # First pair: run both halves of st=0 (gated only
                            # on the small prefetched xa chunk) before st=1
                            # (gated on the full first band DMA).
                            ots = [
                                opool.tile([128, 2 * S], mybir.dt.float32, name="ot")
                                for _ in range(2)
                            ]
                            for j, half in [(0, 0), (0, 1), (1, 0), (1, 1)]:
                                emit_group(n, b, sts[j], half, ots[half], j)
                            for half in range(2):
                                emit_dma(n, b, half, sts, ots[half])
                        else:
                            for half in range(2):
                                ot = opool.tile(
                                    [128, 2 * S], mybir.dt.float32, name="ot"
                                )
                                for j, st in enumerate(sts):
                                    emit_group(n, b, st, half, ot, j)
                                emit_dma(n, b, half, sts, ot)
    nc.compile()
    return nc


def _get_nc():
    global _nc_cache
    if _nc_cache is None:
        _nc_cache = _build()
    return _nc_cache


def kernel(x, kernel):
    global LAST_RESULT
    x = np.asarray(x)
    k = np.asarray(kernel)

    # wt[ci, half, tap*128 + co'] = sign(kernel[dh, dw, ci, half*128 + co'])
    wb = np.where(k >= 0, np.float16(1), np.float16(-1))  # [3,3,128,256]
    wt = np.ascontiguousarray(
        wb.transpose(2, 0, 1, 3)          # [ci, dh, dw, co]
        .reshape(CI, 9, 2, 128)           # co -> (half, co')
        .transpose(0, 2, 1, 3)            # [ci, half, tap, co']
        .reshape(CI, 2, 9 * 128)
    )

    x16 = x.astype(np.float16)  # [32,112,112,128]
    in_maps = []
    for c in range(N_CORES):
        xp = np.zeros((CI, NPC, HP, WP), np.float16)
        xp[:, :, 1:113, 1:113] = x16[c * NPC : (c + 1) * NPC].transpose(3, 0, 1, 2)
        in_maps.append({"xp": xp, "wt": wt})

    nc = _get_nc()
    trace = os.environ.get("BCONV_TRACE", "0") == "1"
    kwargs = {}
    if trace and os.environ.get("BCONV_TRACE_CORES", "") == "all":
        kwargs["trace_cores"] = list(range(N_CORES))
    res = run_bass_kernel_spmd(
        nc, in_maps, core_ids=list(range(N_CORES)), trace=trace, **kwargs
    )
    LAST_RESULT = res

    out = np.empty((32, H, H, CO), np.float32)
    for c in range(N_CORES):
        o = res.results[c]["out_cm"].reshape(CO, NPC, H, H)
        out[c * NPC : (c + 1) * NPC] = o.transpose(1, 2, 3, 0)
    return out



# revision 2
# speedup vs baseline: 1.0613x; 1.0613x over previous
"""BinaryConnect 3x3 SAME conv (NHWC, 32x112x112x128 -> 32x112x112x256) on 8 trn2 cores.

Strategy (data-parallel, 4 images per core, fp8 DoubleRow):
  - Host: binarize kernel to +/-1 fp8e4 (exact). Quantize x to e4m3 (x8) plus
    an e4m3 residual r8 = e4m3(x - x8); store channel-major with a 1-px zero
    halo as two planes [cin, n, {x8,r8}, hp, wp=128].
  - Device: each output tile (4 rows x 112 cols, one cout half) is 7
    accumulating DoubleRow matmuls (2 fp8 MACs/cell/cycle):
      * 2 "pair" slots: taps (0,dw)+(1,dw) for dw in {0,1} share one slot via
        an overlapping 4D AP (k-tile dim steps one image row).
      * 5 "correction" slots: tap t computed as w*x8[t] + w*r8[t] (k-tile dim
        steps across the x8/r8 planes), making those taps ~fp16-exact.
    Quantization error lands only on the 4 paired taps: rel err ~1.8e-2.
  - Output fp16 (halves write traffic), un-transposed + cast to fp32 on host.
"""

import os

import numpy as np
import ml_dtypes

import concourse.bass as bass
import concourse.mybir as mybir
import concourse.tile as tile
from concourse import bacc
from concourse.bass_utils import run_bass_kernel_spmd

N_CORES = 8
NPC = 4            # images per core
H = 112
WP = 128           # padded row width (16B-aligned for DoubleRow k-tile steps)
HP = 115           # 1 top pad + 112 rows + 1 bottom pad + 1 zero tail row
CI = 128
CO = 256
TROWS = 4          # output rows per matmul tile
S = TROWS * H      # 448 matmul free dim (<=512 fp32 PSUM bank)
BROWS = 28         # output rows per input band
NB = H // BROWS    # 4 bands per image
BIN = BROWS + 3    # input rows per band incl. halo
TSB = BROWS // TROWS  # 7 tiles per band

# 7 DoubleRow slots per output tile:
PAIRS = [(0, 1), (1, 1)]            # (dw, n/a): taps (0,dw)+(1,dw)
PAIR_DWS = [0, 1]
CORR = [(2, 0), (2, 1), (0, 2), (1, 2), (2, 2)]
NSLOTS = len(PAIR_DWS) + len(CORR)  # 7

F8 = ml_dtypes.float8_e4m3
DR = mybir.MatmulPerfMode.DoubleRow

_nc_cache = None
LAST_RESULT = None


def _build():
    nc = bacc.Bacc(
        "TRN2",
        target_bir_lowering=False,
        debug=False,
        num_devices=N_CORES,
    )
    x_d = nc.dram_tensor(
        "xp", [CI, NPC, 2, HP, WP], mybir.dt.float8e4, kind="ExternalInput"
    )
    w_d = nc.dram_tensor(
        "wt", [CI, 2, NSLOTS, 2, 128], mybir.dt.float8e4, kind="ExternalInput"
    )
    o_d = nc.dram_tensor(
        "out_cm", [CO, NPC, H * H], mybir.dt.float16, kind="ExternalOutput"
    )
    with tile.TileContext(nc) as tc:
        with (
            tc.tile_pool(name="xpool", bufs=1) as xpool,
            tc.tile_pool(name="wpool", bufs=1) as wpool,
            tc.tile_pool(name="psum", bufs=8, space=bass.MemorySpace.PSUM) as psum,
            tc.tile_pool(name="opool", bufs=12) as opool,
        ):
            # Warmup operand with no DMA dependency: memset, so the PE warmup
            # (HAM un-throttle) can start right after the framework preamble,
            # overlapping the input DMA latency.
            wta = wpool.tile([CI, 2, S], mybir.dt.float8e4, tag="wta", name="wta")
            nc.gpsimd.memset(wta[:], 0.0)
            # Weights split by cout half (separate tiles, so the first matmul
            # group gates on a tiny DMA); first-chunk input in between.
            wt0 = wpool.tile(
                [CI, NSLOTS, 2, 128], mybir.dt.float8e4, tag="wt0", name="wt0"
            )
            nc.sync.dma_start(wt0[:], w_d[:, 0, :, :, :])
            # Small first chunk of image 0 (rows 0-11, both planes) so the
            # first real matmul group (st=0/1) gates on a small DMA instead of
            # a full band. ACT ring keeps its receipt off the sync ring.
            xa = xpool.tile([CI, 2, 12, WP], mybir.dt.float8e4, tag="xa", name="xa")
            nc.scalar.dma_start(xa[:], x_d[:, 0, :, 0:12, :])
            wt1 = wpool.tile(
                [CI, NSLOTS, 2, 128], mybir.dt.float8e4, tag="wt1", name="wt1"
            )
            nc.sync.dma_start(wt1[:], w_d[:, 1, :, :, :])
            wt_h = [wt0, wt1]
            # PE warmup: throwaway DR matmuls to push the HAM activity window
            # to K=8/8 before the real stream begins.
            wu = psum.tile([128, S], mybir.dt.float32, name="ps")
            for _ in range(9):
                nc.tensor.matmul(
                    wu[:], wta[:, :, 0:128], wta[:, :, 0:S],
                    start=True, stop=True, perf_mode=DR,
                )
            # Image 0 is band-split (4 bands of 28 output rows, 31 input rows
            # each incl. halo) so early compute gates on ~1MB chunks.
            # Images 1-3 arrive long before they're needed: one DMA each.
            xs = {}
            for b in range(NB):
                xt = xpool.tile(
                    [CI, 2, BIN, WP], mybir.dt.float8e4, tag=f"x0_{b}", name=f"x0_{b}"
                )
                nc.sync.dma_start(xt[:], x_d[:, 0, :, b * BROWS : b * BROWS + BIN, :])
                xs[0, b] = xt
            for n in range(1, NPC):
                xt = xpool.tile(
                    [CI, 2, HP, WP], mybir.dt.float8e4, tag=f"xi{n}", name=f"xi{n}"
                )
                nc.sync.dma_start(xt[:], x_d[:, n, :, :, :])
                for b in range(NB):
                    xs[n, b] = xt
            # Spatial tiles are processed in pairs per output DMA (bigger
            # per-partition contiguous runs -> fewer SDMA packets).
            pairs = [(0, 1), (2, 3), (4, 5), (6,)]

            def emit_group(n, b, st, half, ot, j):
                if n == 0:
                    r0 = st * TROWS  # band-relative top output row
                    xsrc = xa if b == 0 and st <= 1 else xs[n, b]
                else:
                    r0 = b * BROWS + st * TROWS  # image-relative row
                    xsrc = xs[n, b]
                ps = psum.tile([128, S], mybir.dt.float32, name="ps")
                s = 0
                for dw in PAIR_DWS:
                    # overlapping 4D AP in the x8 plane: k-tile j steps one
                    # image row (tap dh=j), then [4 rows, 112 cols]
                    nat = xsrc[:, 0:2, r0 : r0 + TROWS, dw : dw + H]
                    pstep = nat.ap[0][0]
                    rhs = bass.AP(
                        nat.tensor, r0 * WP + dw,
                        [[pstep, CI], [WP, 2], [WP, TROWS], [1, H]],
                    )
                    nc.tensor.matmul(
                        ps[:], wt_h[half][:, s, :, :], rhs,
                        start=(s == 0), stop=(s == NSLOTS - 1), perf_mode=DR,
                    )
                    s += 1
                for (dh, dw) in CORR:
                    rhs = xsrc[:, 0:2, r0 + dh : r0 + dh + TROWS, dw : dw + H]
                    nc.tensor.matmul(
                        ps[:], wt_h[half][:, s, :, :], rhs,
                        start=(s == 0), stop=(s == NSLOTS - 1), perf_mode=DR,
                    )
                    s += 1
                nc.vector.tensor_copy(ot[:, j * S : (j + 1) * S], ps[:])

            def emit_dma(n, b, half, sts, ot):
                width = len(sts) * S
                o0 = (b * BROWS + sts[0] * TROWS) * H
                # ACT's HWDGE ring — keeps output DMAs off the sync ring so
                # they don't queue behind input DMAs.
                nc.scalar.dma_start(
                    o_d[half * 128 : half * 128 + 128, n, o0 : o0 + width],
                    ot[:, 0:width],
                )

            for n in range(NPC):
                for b in range(NB):
                    for sts in pairs:
                        if (n, b, sts) == (0, 0, (0, 1)):
                            # First pair: run both halves of st=0 (gated only
                            # on the small prefetched xa chunk) before st=1.
                            ots = [
                                opool.tile([128, 2 * S], mybir.dt.float16, name="ot")
                                for _ in range(2)
                            ]
                            for j, half in [(0, 0), (0, 1), (1, 0), (1, 1)]:
                                emit_group(n, b, sts[j], half, ots[half], j)
                            for half in range(2):
                                emit_dma(n, b, half, sts, ots[half])
                        else:
                            for half in range(2):
                                ot = opool.tile(
                                    [128, 2 * S], mybir.dt.float16, name="ot"
                                )
                                for j, st in enumerate(sts):
                                    emit_group(n, b, st, half, ot, j)
                                emit_dma(n, b, half, sts, ot)
    nc.compile()
    return nc


def _get_nc():
    global _nc_cache
    if _nc_cache is None:
        _nc_cache = _build()
    return _nc_cache


def kernel(x, kernel):
    global LAST_RESULT
    x = np.asarray(x)
    k = np.asarray(kernel)

    # wb[dh,dw,ci,co] = sign(kernel) in {+1,-1}; slot layout
    # wt[ci, half, slot, jslot, co'] with jslot = k-tile index.
    wb = np.where(k >= 0, np.float32(1), np.float32(-1))  # [3,3,128,256]
    wt = np.zeros((CI, 2, NSLOTS, 2, 128), np.float32)
    for half in range(2):
        co = slice(half * 128, half * 128 + 128)
        s = 0
        for dw in PAIR_DWS:
            wt[:, half, s, 0, :] = wb[0, dw, :, co]
            wt[:, half, s, 1, :] = wb[1, dw, :, co]
            s += 1
        for (dh, dw) in CORR:
            wt[:, half, s, 0, :] = wb[dh, dw, :, co]
            wt[:, half, s, 1, :] = wb[dh, dw, :, co]
            s += 1
    wt = np.ascontiguousarray(wt.astype(F8))

    # e4m3 main + e4m3 residual quantization (both exact on device)
    x8 = x.astype(F8)
    r8 = (x - x8.astype(np.float32)).astype(F8)

    in_maps = []
    for c in range(N_CORES):
        xp = np.zeros((CI, NPC, 2, HP, WP), F8)
        sl = slice(c * NPC, (c + 1) * NPC)
        xp[:, :, 0, 1:113, 1:113] = x8[sl].transpose(3, 0, 1, 2)
        xp[:, :, 1, 1:113, 1:113] = r8[sl].transpose(3, 0, 1, 2)
        in_maps.append({"xp": xp, "wt": wt})

    nc = _get_nc()
    trace = os.environ.get("BCONV_TRACE", "0") == "1"
    kwargs = {}
    if trace and os.environ.get("BCONV_TRACE_CORES", "") == "all":
        kwargs["trace_cores"] = list(range(N_CORES))
    res = run_bass_kernel_spmd(
        nc, in_maps, core_ids=list(range(N_CORES)), trace=trace, **kwargs
    )
    LAST_RESULT = res

    out = np.empty((32, H, H, CO), np.float32)
    for c in range(N_CORES):
        o = res.results[c]["out_cm"].reshape(CO, NPC, H, H).astype(np.float32)
        out[c * NPC : (c + 1) * NPC] = o.transpose(1, 2, 3, 0)
    return out
